# revision 1
# baseline (speedup 1.0000x reference)
"""Trainium2 Bass kernel for a dense transformer block (RMSNorm + RoPE
causal/text-mask attention + RMSNorm + SwiGLU MLP, two residuals).

Distribution: data-parallel over batch (B=2) x query-row-parallel over
token tiles (4 ways) => 8 cores, zero collectives. Each core receives the
full batch element's x (to compute K/V for the whole sequence, replicated
within its group of 4) plus a gathered set of 4 query-row tiles
(interleaved round-robin so causal work is balanced), and computes the
final output rows for exactly those query tiles. The host scatters the 8
(512, 1024) results back into the (2, 2048, 1024) output.

All matmuls run as float32r (TF32-like full-rate PE path, fp32 storage).
Softmax skips the max-subtraction (logits are ~N(0,1) by construction);
the denominator comes free from an appended ones-column on V.
"""

import numpy as np

import concourse.bass as bass
import concourse.mybir as mybir
import concourse.tile as tile
from concourse.bass_utils import run_bass_kernel_spmd
from concourse.masks import make_identity
from concourse.vector_clock import ScopedClock

# ---------------------------------------------------------------- shapes
B = 2
T = 2048
C = 1024
NH = 16
HD = 64
NHID = 2816
EPS = 1e-5
P = 128
TTILES = T // P      # 16
QTILES = 4           # q-row tiles per core
QROWS = QTILES * P   # 512
CCH = C // P         # 8 contraction chunks
HSB = NHID // P      # 22 hidden blocks
F32 = mybir.dt.float32
F32R = mybir.dt.float32r
NEG = -1.0e30

# ------------------------------------------------- TileContext drain patch
# This walrus build rejects >1 sync-wait on one SP Drain instruction
# ("Too many sync wait commands"). Split the final Tile drain's waits
# across chained drains, one wait each.
_DRAIN_CAP = 1


def _patched_drain_and_barrier(self, tick_clock, wait_clock):
    nc = self.nc
    drain_inst = nc.sync.drain()
    wait_clock.add_sem_waits(
        drain_inst.ins, ScopedClock({None: tick_clock.global_clock})
    )
    waits = list(drain_inst.ins.sync_info.on_wait)
    if len(waits) > _DRAIN_CAP:
        upd = list(drain_inst.ins.sync_info.on_update)
        drain_inst.ins.sync_info = mybir.SyncInfo(
            on_wait=waits[:_DRAIN_CAP], on_update=upd
        )
        rest = waits[_DRAIN_CAP:]
        while rest:
            d2 = nc.sync.drain()
            d2.ins.sync_info = mybir.SyncInfo(
                on_wait=rest[:_DRAIN_CAP], on_update=[]
            )
            rest = rest[_DRAIN_CAP:]
    nc.all_engine_barrier()
    popped = nc._tile_sem_poison_stack.pop()
    assert popped is self._sem_poison
    nc.clear_and_free_semaphores(list(self.sems.allocated().values()))
    nc.all_engine_barrier()


tile.TileContext._drain_and_barrier = _patched_drain_and_barrier

# This walrus also rejects >1 sync-wait on other instruction encodings
# (LDWEIGHTS, DMA triggers, ...). Post-process the serialized BIR before
# walrus: any instruction with more than one wait keeps its last wait and
# the rest move to NoOp instructions inserted just before it on the same
# engine (waits-before-exec on the same engine stream is always safe).
_JSON_WAIT_CAP = 1
_WAIT_CAPS = {}
_WAIT_CAP_DEFAULT = 1


def _split_bir_waits(bir_bytes):
    import orjson
    d = orjson.loads(bir_bytes)
    n_split = 0
    for fn in d["functions"]:
        for blk in fn["blocks"]:
            out = []
            for inst in blk["instructions"]:
                si = inst.get("sync_info")
                waits = (si or {}).get("on_wait") or []
                cap = _WAIT_CAPS.get(inst.get("opcode"), _WAIT_CAP_DEFAULT)
                if len(waits) > cap:
                    keep = waits[-cap:]
                    extra = waits[:-cap]
                    w_i = 0
                    while extra:
                        chunk = extra[:_JSON_WAIT_CAP]
                        extra = extra[_JSON_WAIT_CAP:]
                        nop = {
                            "name": f"{inst['name']}_w{w_i}",
                            "opcode": "NoOp",
                            "engine": inst["engine"],
                            "ins": [],
                            "outs": [],
                            "sync_info": {"on_wait": chunk, "on_update": []},
                        }
                        if "debug" in inst:
                            nop["debug"] = inst["debug"]
                        out.append(nop)
                        w_i += 1
                    si["on_wait"] = keep
                    n_split += 1
                out.append(inst)
            blk["instructions"] = out
    return orjson.dumps(d), n_split


import concourse.bass_utils as _bass_utils_mod
import concourse.bass2jax as _bass2jax_mod

_orig_compile_bir_kernel = _bass_utils_mod.compile_bir_kernel


def _patched_compile_bir_kernel(ant_bir_str, compile_dir_path, **kwargs):
    fixed, n = _split_bir_waits(ant_bir_str)
    return _orig_compile_bir_kernel(fixed, compile_dir_path, **kwargs)


_bass_utils_mod.compile_bir_kernel = _patched_compile_bir_kernel
_bass2jax_mod.compile_bir_kernel = _patched_compile_bir_kernel


# ------------------------------------------------------------ device code
def _rope6(nc, pool, src_psum, dst_sbuf, cos_sb, sin_sb, tag):
    """RoPE on a (128, 1024) tile: interleaved pairs along the free dim.
    src_psum (128,1024) PSUM, dst_sbuf (128,1024) SBUF.
    cos_sb/sin_sb: (128, 512) expanded per-pair factors."""
    s3 = src_psum.rearrange("p (i two) -> p i two", two=2)
    d3 = dst_sbuf.rearrange("p (i two) -> p i two", two=2)
    e, o = s3[:, :, 0], s3[:, :, 1]
    t1 = pool.tile([P, 512], F32, tag=f"{tag}_t1", name=f"{tag}_t1")
    t2 = pool.tile([P, 512], F32, tag=f"{tag}_t2", name=f"{tag}_t2")
    nc.vector.tensor_mul(t1, e, cos_sb)
    nc.vector.tensor_mul(t2, o, sin_sb)
    nc.vector.tensor_sub(d3[:, :, 0], t1, t2)
    nc.vector.tensor_mul(t1, e, sin_sb)
    nc.vector.tensor_mul(t2, o, cos_sb)
    nc.vector.tensor_add(d3[:, :, 1], t1, t2)


def _rmsnorm_factor(nc, statp, scratch, x_sb, eps_t, tag):
    """Return (128,1) SBUF tile with 1/sqrt(mean(x^2)+eps) for x_sb (128,C)."""
    ssq = statp.tile([P, 1], F32, tag=f"{tag}_ssq", name=f"{tag}_ssq")
    nc.scalar.activation(
        out=scratch, in_=x_sb, func=mybir.ActivationFunctionType.Square,
        accum_out=ssq,
    )
    f = statp.tile([P, 1], F32, tag=f"{tag}_f", name=f"{tag}_f")
    nc.scalar.activation(
        out=f, in_=ssq, func=mybir.ActivationFunctionType.Sqrt,
        bias=eps_t, scale=1.0 / C,
    )
    nc.vector.reciprocal(f, f)
    return f


def _emit(ctx_pools, tc, nc, prm, upto='full'):
    """Emit the whole per-core program. prm: dict of bass APs."""
    from contextlib import ExitStack

    def _dummy_out():
        nc.sync.dma_start(out=prm["out"][:, :], in_=prm["xq"][:, :])

    with ExitStack() as es:
        constp = es.enter_context(tc.tile_pool(name="const", bufs=1))
        identity_f = constp.tile([P, P], F32, tag="identity_f",
                                 name="identity_f")
        make_identity(nc, identity_f)
        identity = constp.tile([P, P], F32R, tag="identity", name="identity")
        nc.vector.tensor_copy(identity, identity_f)
        eps_t = constp.tile([P, 1], F32, tag="eps", name="eps")
        nc.vector.memset(eps_t, EPS)
        ones_f = constp.tile([P, HD], F32, tag="ones_f", name="ones_f")
        nc.vector.memset(ones_f, 1.0)
        ones_full = constp.tile([P, HD], F32R, tag="ones_full",
                                name="ones_full")
        nc.vector.tensor_copy(ones_full, ones_f)

        # persistent SBUF arrays (packed 2 heads per 128 partitions)
        qTp = es.enter_context(tc.tile_pool(name="qT", bufs=1))
        QTm = qTp.tile([P, CCH, QROWS], F32R, tag="qTm", name="qTm")
        QT = [QTm[:, c, :] for c in range(CCH)]
        yT2p = es.enter_context(tc.tile_pool(name="yT2", bufs=1))
        yT2 = [yT2p.tile([P, QROWS], F32R, tag=f"yT2{c}", name=f"yT2{c}")
               for c in range(CCH)]

        # DRAM scratch (through a DRAM pool so Tile tracks the RAW deps)
        dramp = es.enter_context(tc.tile_pool(name="dram", bufs=1, space="DRAM"))
        KTd = dramp.tile([C, T], F32, tag="KTd", name="KTd")
        Vd = dramp.tile([T, NH * (HD + 1)], F32, tag="Vd", name="Vd")

        with tc.tile_pool(name="hT", bufs=1) as hTp:
            hTm = hTp.tile([P, CCH, T], F32R, tag="hTm", name="hTm")
            hT = [hTm[:, c, :] for c in range(CCH)]

            with tc.tile_pool(name="hqT", bufs=1) as hqTp:
                hqTm = hqTp.tile([P, CCH, QROWS], F32R, tag="hqTm",
                                 name="hqTm")
                hqT = [hqTm[:, c, :] for c in range(CCH)]

                # ------- P1: hT = rmsnorm(xf).T ; hqT = rmsnorm(xq).T ----
                with tc.tile_pool(name="p1", bufs=3) as p1, \
                     tc.tile_pool(name="stat", bufs=4) as statp, \
                     tc.tile_pool(name="tp1", bufs=3, space="PSUM") as tpp:
                    scratch = p1.tile([P, C], F32, tag="sq_scratch",
                                      name="sq_scratch", bufs=1)
                    for src, n_t, dstm in ((prm["xf"], TTILES, hTm),
                                           (prm["xq"], QTILES, hqTm)):
                        dstT = [dstm[:, :, tt * P:(tt + 1) * P]
                                for tt in range(n_t)]
                        for tt in range(n_t):
                            xt = p1.tile([P, C], F32, tag="xt", name="xt")
                            nc.sync.dma_start(
                                out=xt, in_=src[tt * P:(tt + 1) * P, :])
                            f = _rmsnorm_factor(nc, statp, scratch, xt,
                                                eps_t, "n1")
                            ht = p1.tile([P, C], F32R, tag="ht", name="ht")
                            nc.vector.tensor_scalar_mul(ht, xt, f)
                            tp = tpp.tile([P, CCH, P], F32R, tag="tp",
                                          name="tp")
                            for dc in range(CCH):
                                nc.tensor.transpose(
                                    tp[:, dc, :],
                                    ht[:, dc * P:(dc + 1) * P],
                                    identity)
                            nc.vector.tensor_copy(
                                dstT[tt], tp)

                if upto == 'p1':
                    _dummy_out()
                    return
                # ------- P2c: QT = rope(hq @ Wq).T -----------------------
                with tc.tile_pool(name="wq", bufs=1) as wqp, \
                     tc.tile_pool(name="p2q", bufs=2) as p2q, \
                     tc.tile_pool(name="qps", bufs=2, space="PSUM") as qpp, \
                     tc.tile_pool(name="tp2q", bufs=2, space="PSUM") as tpq:
                    wq = [wqp.tile([P, C], F32R, tag=f"wq{c}", name=f"wq{c}")
                          for c in range(CCH)]
                    for cc in range(CCH):
                        nc.sync.dma_start(
                            out=wq[cc], in_=prm["Wq"][cc * P:(cc + 1) * P, :].bitcast(F32R))
                    for qt in range(QTILES):
                        qp = qpp.tile([P, C], F32, tag="qpsum", name="qpsum")
                        for half in range(2):
                            for cc in range(CCH):
                                nc.tensor.matmul(
                                    qp[:, half * 512:(half + 1) * 512],
                                    lhsT=hqT[cc][:, qt * P:(qt + 1) * P],
                                    rhs=wq[cc][:, half * 512:(half + 1) * 512],
                                    start=(cc == 0), stop=(cc == CCH - 1),
                                )
                        cq = p2q.tile([P, 512], F32, tag="cosq", name="cosq")
                        sq = p2q.tile([P, 512], F32, tag="sinq", name="sinq")
                        nc.sync.dma_start(
                            out=cq, in_=prm["cos_q"][qt * P:(qt + 1) * P, :])
                        nc.sync.dma_start(
                            out=sq, in_=prm["sin_q"][qt * P:(qt + 1) * P, :])
                        qs = p2q.tile([P, C], F32R, tag="qstage", name="qstage")
                        _rope6(nc, p2q, qp, qs, cq, sq, "rq")
                        tpw = tpq.tile([P, CCH, P], F32R, tag="tpq",
                                       name="tpq")
                        for dc in range(CCH):
                            nc.tensor.transpose(
                                tpw[:, dc, :],
                                qs[:, dc * P:(dc + 1) * P],
                                identity)
                        nc.vector.tensor_copy(
                            QTm[:, :, qt * P:(qt + 1) * P], tpw)

            # ------- P2a: KTd = rope(h @ Wk).T (via DRAM) ---------------
            with tc.tile_pool(name="wk", bufs=1) as wkp, \
                 tc.tile_pool(name="p2k", bufs=2) as p2k, \
                 tc.tile_pool(name="kts", bufs=3) as ktsp, \
                 tc.tile_pool(name="kps", bufs=2, space="PSUM") as kpp, \
                 tc.tile_pool(name="tp2k", bufs=2, space="PSUM") as tpk:
                wk = [wkp.tile([P, C], F32R, tag=f"wk{c}", name=f"wk{c}")
                      for c in range(CCH)]
                for cc in range(CCH):
                    nc.sync.dma_start(
                        out=wk[cc], in_=prm["Wk"][cc * P:(cc + 1) * P, :].bitcast(F32R))
                for tt in range(TTILES):
                    kp = kpp.tile([P, C], F32, tag="kpsum", name="kpsum")
                    for half in range(2):
                        for cc in range(CCH):
                            nc.tensor.matmul(
                                kp[:, half * 512:(half + 1) * 512],
                                lhsT=hT[cc][:, tt * P:(tt + 1) * P],
                                rhs=wk[cc][:, half * 512:(half + 1) * 512],
                                start=(cc == 0), stop=(cc == CCH - 1),
                            )
                    ck = p2k.tile([P, 512], F32, tag="cosk", name="cosk")
                    sk = p2k.tile([P, 512], F32, tag="sink", name="sink")
                    nc.sync.dma_start(
                        out=ck, in_=prm["cos_k"][tt * P:(tt + 1) * P, :])
                    nc.sync.dma_start(
                        out=sk, in_=prm["sin_k"][tt * P:(tt + 1) * P, :])
                    ks = p2k.tile([P, C], F32R, tag="kstage", name="kstage")
                    _rope6(nc, p2k, kp, ks, ck, sk, "rk")
                    tpw = tpk.tile([P, CCH, P], F32R, tag="tpk", name="tpk")
                    for dc in range(CCH):
                        nc.tensor.transpose(
                            tpw[:, dc, :],
                            ks[:, dc * P:(dc + 1) * P],
                            identity)
                    kts = ktsp.tile([P, CCH, P], F32R, tag="ktstage",
                                    name="ktstage")
                    nc.vector.tensor_copy(kts, tpw)
                    nc.sync.dma_start(
                        out=KTd.rearrange("(dc p) t -> p dc t", p=P)[
                            :, :, tt * P:(tt + 1) * P].bitcast(F32R),
                        in_=kts)

            # ------- P2b: Vd = (h @ Wv | ones) (via DRAM) ---------------
            with tc.tile_pool(name="wv", bufs=1) as wvp, \
                 tc.tile_pool(name="p2v", bufs=3) as p2v, \
                 tc.tile_pool(name="vps", bufs=2, space="PSUM") as vpp:
                wv = [wvp.tile([P, C], F32R, tag=f"wv{c}", name=f"wv{c}")
                      for c in range(CCH)]
                for cc in range(CCH):
                    nc.sync.dma_start(
                        out=wv[cc], in_=prm["Wv"][cc * P:(cc + 1) * P, :].bitcast(F32R))
                for tt in range(TTILES):
                    vp = vpp.tile([P, C], F32, tag="vpsum", name="vpsum")
                    for half in range(2):
                        for cc in range(CCH):
                            nc.tensor.matmul(
                                vp[:, half * 512:(half + 1) * 512],
                                lhsT=hT[cc][:, tt * P:(tt + 1) * P],
                                rhs=wv[cc][:, half * 512:(half + 1) * 512],
                                start=(cc == 0), stop=(cc == CCH - 1),
                            )
                    vsb = p2v.tile([P, NH * (HD + 1)], F32, tag="vsb",
                                   name="vsb")
                    v3 = vsb.rearrange("p (h e) -> p h e", e=HD + 1)
                    nc.vector.tensor_copy(
                        v3[:, :, 0:HD],
                        vp.rearrange("p (h d) -> p h d", d=HD))
                    nc.vector.memset(v3[:, :, HD:HD + 1], 1.0)
                    nc.sync.dma_start(
                        out=Vd[tt * P:(tt + 1) * P, :], in_=vsb)

        if upto == 'p2':
            _dummy_out()
            return
        # ---------------- P3: attention per head ------------------------
        with tc.tile_pool(name="maskp", bufs=1) as maskp, \
             tc.tile_pool(name="kth", bufs=2) as kthp, \
             tc.tile_pool(name="vh", bufs=2) as vhp, \
             tc.tile_pool(name="pt", bufs=3) as ptp, \
             tc.tile_pool(name="rdp", bufs=2) as rdp, \
             tc.tile_pool(name="stps", bufs=3, space="PSUM") as stpp, \
             tc.tile_pool(name="ytps", bufs=2, space="PSUM") as ytpp, \
             tc.tile_pool(name="rbps", bufs=2, space="PSUM") as rbpp:
            maskT = [maskp.tile([P, QROWS], F32, tag=f"mask{kt}",
                                name=f"mask{kt}") for kt in range(TTILES)]
            for kt in range(TTILES):
                nc.sync.dma_start(
                    out=maskT[kt], in_=prm["maskT"][kt * P:(kt + 1) * P, :])
            for dc in range(CCH):
                kth2 = kthp.tile([P, T], F32R, tag="kth2", name="kth2")
                nc.sync.dma_start(out=kth2,
                                  in_=KTd[dc * P:(dc + 1) * P, :].bitcast(F32R))
                vh2 = vhp.tile([P, TTILES, 2 * (HD + 1)], F32R, tag="vh2",
                               name="vh2")
                nc.sync.dma_start(
                    out=vh2,
                    in_=Vd[:, dc * 2 * (HD + 1):(dc + 1) * 2 * (HD + 1)]
                    .rearrange("(kt p) e -> p kt e", p=P).bitcast(F32R))
                for hh in range(2):
                    h = dc * 2 + hh
                    sub = hh * HD
                    vh = vh2[:, :, hh * (HD + 1):(hh + 1) * (HD + 1)]
                    ytp = ytpp.tile([HD + 1, QROWS], F32, tag="ytp",
                                    name="ytp")
                    for kt in range(TTILES):
                        # With q-tiles assigned round-robin (core t owns
                        # absolute tiles {t, t+4, t+8, t+12}), q-tile j of
                        # every core sits at absolute tile >= 4j, so k-tile
                        # kt is strictly above the causal diagonal for all
                        # q-tiles j < kt//4 (skip), fully allowed for
                        # j > kt//4 (exp straight from PSUM), and needs the
                        # data mask only on the single stripe j == kt//4.
                        jm = kt // 4
                        qo = jm * P
                        w = QROWS - qo
                        stp = stpp.tile([P, QROWS], F32, tag="stp",
                                        name="stp")
                        nc.tensor.matmul(
                            stp[:, qo:],
                            lhsT=kth2[sub:sub + HD,
                                      kt * P:(kt + 1) * P],
                            rhs=QT[dc][sub:sub + HD, qo:],
                            start=True, stop=True,
                        )
                        pt = ptp.tile([P, QROWS], F32R, tag="pt", name="pt")
                        nc.vector.tensor_add(
                            pt[:, qo:qo + P], stp[:, qo:qo + P],
                            maskT[kt][:, qo:qo + P])
                        nc.scalar.activation(
                            out=pt[:, qo:qo + P], in_=pt[:, qo:qo + P],
                            func=mybir.ActivationFunctionType.Exp,
                            scale=0.125)
                        if w > P:
                            nc.scalar.activation(
                                out=pt[:, qo + P:], in_=stp[:, qo + P:],
                                func=mybir.ActivationFunctionType.Exp,
                                scale=0.125)
                        nc.tensor.matmul(
                            ytp[:, qo:],
                            lhsT=vh[:, kt, :],
                            rhs=pt[:, qo:],
                            start=(kt == 0), stop=(kt == TTILES - 1),
                            skip_group_check=True,
                        )
                    den = rdp.tile([HD + 1, QROWS], F32, tag="den",
                                   name="den")
                    nc.vector.tensor_copy(den[HD:HD + 1, :],
                                          ytp[HD:HD + 1, :])
                    nc.vector.reciprocal(den[HD:HD + 1, :],
                                         den[HD:HD + 1, :])
                    denr = rdp.tile([HD + 1, QROWS], F32R, tag="denr",
                                    name="denr")
                    nc.vector.tensor_copy(denr[HD:HD + 1, :],
                                          den[HD:HD + 1, :])
                    rdb = rbpp.tile([HD, QROWS], F32, tag="rdb", name="rdb")
                    nc.tensor.matmul(
                        rdb, lhsT=ones_full[HD:HD + 1, :],
                        rhs=denr[HD:HD + 1, :],
                        start=True, stop=True)
                    rdbs = rdp.tile([HD, QROWS], F32, tag="rdbs",
                                    name="rdbs")
                    nc.vector.tensor_copy(rdbs, rdb)
                    yn = rdp.tile([HD, QROWS], F32, tag="yn", name="yn")
                    nc.vector.tensor_tensor(
                        out=yn, in0=ytp[0:HD, :], in1=rdbs,
                        op=mybir.AluOpType.mult)
                    nc.sync.dma_start(out=yT2[dc][sub:sub + HD, :],
                                      in_=yn.bitcast(F32R))

        if upto == 'p3':
            _dummy_out()
            return
        # ---------------- P4/P5/P6 --------------------------------------
        with tc.tile_pool(name="x2", bufs=1) as x2pool, \
             tc.tile_pool(name="h2T", bufs=1) as h2Tpool:
            x2sb = [x2pool.tile([P, C], F32, tag=f"x2_{q}", name=f"x2_{q}")
                    for q in range(QTILES)]
            h2Tm = h2Tpool.tile([P, CCH, QROWS], F32R, tag="h2Tm",
                                name="h2Tm")
            h2T = [h2Tm[:, c, :] for c in range(CCH)]
            with tc.tile_pool(name="wpj", bufs=1) as wpjp, \
                 tc.tile_pool(name="p4", bufs=2) as p4, \
                 tc.tile_pool(name="stat4", bufs=4) as stat4, \
                 tc.tile_pool(name="x2ps", bufs=2, space="PSUM") as x2pp, \
                 tc.tile_pool(name="tp4", bufs=2, space="PSUM") as tpp4:
                wpj = [wpjp.tile([P, C], F32R, tag=f"wpj{c}", name=f"wpj{c}")
                       for c in range(CCH)]
                for cc in range(CCH):
                    nc.sync.dma_start(
                        out=wpj[cc], in_=prm["Wproj"][cc * P:(cc + 1) * P, :].bitcast(F32R))
                scratch4 = p4.tile([P, C], F32, tag="sq4_scratch",
                                   name="sq4_scratch", bufs=1)
                for qt in range(QTILES):
                    x2p = x2pp.tile([P, C], F32, tag="x2psum", name="x2psum")
                    for half in range(2):
                        for cc in range(CCH):
                            nc.tensor.matmul(
                                x2p[:, half * 512:(half + 1) * 512],
                                lhsT=yT2[cc][:, qt * P:(qt + 1) * P],
                                rhs=wpj[cc][:, half * 512:(half + 1) * 512],
                                start=(cc == 0), stop=(cc == CCH - 1),
                            )
                    xqt = p4.tile([P, C], F32, tag="xqt", name="xqt")
                    nc.sync.dma_start(
                        out=xqt, in_=prm["xq"][qt * P:(qt + 1) * P, :])
                    nc.vector.tensor_add(x2sb[qt], x2p, xqt)
                    f2 = _rmsnorm_factor(nc, stat4, scratch4, x2sb[qt],
                                         eps_t, "n2")
                    h2 = p4.tile([P, C], F32R, tag="h2stage", name="h2stage")
                    nc.vector.tensor_scalar_mul(h2, x2sb[qt], f2)
                    tpw = tpp4.tile([P, CCH, P], F32R, tag="tp4", name="tp4")
                    for dc in range(CCH):
                        nc.tensor.transpose(
                            tpw[:, dc, :],
                            h2[:, dc * P:(dc + 1) * P],
                            identity)
                    nc.vector.tensor_copy(
                        h2Tm[:, :, qt * P:(qt + 1) * P], tpw)

            if upto == 'p4':
                _dummy_out()
                return
            # ---------------- P5: SwiGLU -> mT (22 x (128, 512)) --------
            with tc.tile_pool(name="mt", bufs=1) as mtp:
                mT = [mtp.tile([P, QROWS], F32R, tag=f"mT{i}", name=f"mT{i}")
                      for i in range(HSB)]
                with tc.tile_pool(name="p5w", bufs=3) as p5w, \
                     tc.tile_pool(name="p5s", bufs=3) as p5s, \
                     tc.tile_pool(name="abps", bufs=2, space="PSUM") as abpp:
                    for hs in range(HSB):
                        w1b = p5w.tile([P, CCH, P], F32R, tag="w1b",
                                       name="w1b")
                        nc.sync.dma_start(
                            out=w1b,
                            in_=prm["W1b"][hs].rearrange(
                                "(cc p) j -> p cc j", p=P).bitcast(F32R))
                        w2b = p5w.tile([P, CCH, P], F32R, tag="w2b",
                                       name="w2b")
                        nc.sync.dma_start(
                            out=w2b,
                            in_=prm["W2b"][hs].rearrange(
                                "(cc p) j -> p cc j", p=P).bitcast(F32R))
                        ap_ = abpp.tile([P, QROWS], F32, tag="apsum",
                                        name="apsum")
                        bp_ = abpp.tile([P, QROWS], F32, tag="bpsum",
                                        name="bpsum")
                        for cc in range(CCH):
                            nc.tensor.matmul(
                                ap_, lhsT=w1b[:, cc, :],
                                rhs=h2T[cc],
                                start=(cc == 0), stop=(cc == CCH - 1))
                        for cc in range(CCH):
                            nc.tensor.matmul(
                                bp_, lhsT=w2b[:, cc, :],
                                rhs=h2T[cc],
                                start=(cc == 0), stop=(cc == CCH - 1))
                        sT = p5s.tile([P, QROWS], F32, tag="sT", name="sT")
                        nc.scalar.activation(
                            out=sT, in_=ap_,
                            func=mybir.ActivationFunctionType.Sigmoid)
                        nc.vector.tensor_tensor(
                            out=sT, in0=sT, in1=bp_,
                            op=mybir.AluOpType.mult)
                        nc.vector.tensor_tensor(
                            out=mT[hs], in0=sT, in1=ap_,
                            op=mybir.AluOpType.mult)

                if upto == 'p5':
                    _dummy_out()
                    return
                # ------------ P6: out = x2 + m @ Wmlp -------------------
                with tc.tile_pool(name="p6w", bufs=3) as p6w, \
                     tc.tile_pool(name="p6o", bufs=2) as p6o, \
                     tc.tile_pool(name="x3ps", bufs=1, space="PSUM") as x3pp:
                    x3p = [x3pp.tile([P, C], F32, tag=f"x3_{q}",
                                     name=f"x3_{q}") for q in range(QTILES)]
                    for hs in range(HSB):
                        wmb = p6w.tile([P, C], F32R, tag="wmb", name="wmb")
                        nc.sync.dma_start(
                            out=wmb, in_=prm["Wmlp"][hs * P:(hs + 1) * P, :].bitcast(F32R))
                        for qt in range(QTILES):
                            for half in range(2):
                                nc.tensor.matmul(
                                    x3p[qt][:, half * 512:(half + 1) * 512],
                                    lhsT=mT[hs][:, qt * P:(qt + 1) * P],
                                    rhs=wmb[:, half * 512:(half + 1) * 512],
                                    start=(hs == 0), stop=(hs == HSB - 1),
                                )
                    for qt in range(QTILES):
                        osb = p6o.tile([P, C], F32, tag="osb", name="osb")
                        nc.vector.tensor_add(osb, x3p[qt], x2sb[qt])
                        nc.sync.dma_start(
                            out=prm["out"][qt * P:(qt + 1) * P, :], in_=osb)


def build_bass(upto='full', repeat=1):
    nc = bass.Bass("TRN2", target_bir_lowering=False, debug=False, num_devices=8)
    prm = {}

    def inp(name, shape):
        prm[name] = nc.declare_dram_parameter(name, list(shape), F32,
                                              isOutput=False).ap()

    inp("xf", (T, C))
    inp("xq", (QROWS, C))
    inp("cos_q", (QROWS, 512))
    inp("sin_q", (QROWS, 512))
    inp("cos_k", (T, 512))
    inp("sin_k", (T, 512))
    inp("maskT", (T, QROWS))
    inp("Wq", (C, C))
    inp("Wk", (C, C))
    inp("Wv", (C, C))
    inp("Wproj", (C, C))
    inp("W1b", (HSB, C, P))
    inp("W2b", (HSB, C, P))
    inp("Wmlp", (NHID, C))
    prm["out"] = nc.declare_dram_parameter("out", [QROWS, C], F32,
                                           isOutput=True).ap()
    with tile.TileContext(nc) as tc:
        for r in range(repeat):
            if r == repeat - 1:
                _emit(None, tc, nc, prm, upto=upto)
            else:
                sink = nc.dram_tensor(f"outsink{r}", [QROWS, C], F32).ap()
                _emit(None, tc, nc, dict(prm, out=sink), upto=upto)
    return nc


# ------------------------------------------------------------- host glue
def _rope_tables():
    theta = (1.0 / (10000.0 ** (np.arange(0, HD, 2, dtype=np.float32)
                                / np.float32(HD)))).astype(np.float32)
    ang = np.outer(np.arange(T, dtype=np.float32), theta).astype(np.float32)
    cos = np.cos(ang).astype(np.float32)
    sin = np.sin(ang).astype(np.float32)
    # expand to all heads: col j of the 512-wide table is pair (j % 32)
    return np.tile(cos, (1, NH)), np.tile(sin, (1, NH))


def core_rows(c):
    t = c % 4
    tiles = [t, t + 4, t + 8, t + 12]
    return np.concatenate([np.arange(a * P, (a + 1) * P) for a in tiles])


def make_in_maps(x, y_mask, Wqkv, Wattn_proj, scale1, scale2, Wfc1, Wfc2,
                 Wmlp_proj):
    f = np.float32
    Wq = (scale1[:, None] * Wqkv[:, 0:C]).astype(f)
    Wk = (scale1[:, None] * Wqkv[:, C:2 * C]).astype(f)
    Wv = (scale1[:, None] * Wqkv[:, 2 * C:3 * C]).astype(f)
    Wproj = np.ascontiguousarray(Wattn_proj.astype(f))
    W1f = (scale2[:, None] * Wfc1).astype(f)
    W2f = (scale2[:, None] * Wfc2).astype(f)
    W1b = np.ascontiguousarray(W1f.reshape(C, HSB, P).transpose(1, 0, 2))
    W2b = np.ascontiguousarray(W2f.reshape(C, HSB, P).transpose(1, 0, 2))
    Wmlp = np.ascontiguousarray(Wmlp_proj.astype(f))
    cos_e, sin_e = _rope_tables()

    kidx = np.arange(T)
    in_maps = []
    for c in range(8):
        b = c // 4
        rows = core_rows(c)
        ym = np.zeros(T, bool)
        ym[:64] = y_mask[b].astype(bool)
        text = ym[:, None] & ym[rows][None, :]
        allowed = (kidx[:, None] <= rows[None, :]) | text
        maskT = np.where(allowed, 0.0, NEG).astype(f)
        in_maps.append({
            "xf": np.ascontiguousarray(x[b].astype(f)),
            "xq": np.ascontiguousarray(x[b][rows].astype(f)),
            "cos_q": np.ascontiguousarray(cos_e[rows]),
            "sin_q": np.ascontiguousarray(sin_e[rows]),
            "cos_k": cos_e,
            "sin_k": sin_e,
            "maskT": maskT,
            "Wq": Wq, "Wk": Wk, "Wv": Wv, "Wproj": Wproj,
            "W1b": W1b, "W2b": W2b, "Wmlp": Wmlp,
        })
    return in_maps


_NC_CACHE = None


def kernel(**inputs):
    global _NC_CACHE
    in_maps = make_in_maps(**{k: np.asarray(v) for k, v in inputs.items()})
    if _NC_CACHE is None:
        _NC_CACHE = build_bass()
    res = run_bass_kernel_spmd(_NC_CACHE, in_maps, core_ids=list(range(8)))
    out = np.empty((B, T, C), np.float32)
    for c in range(8):
        out[c // 4, core_rows(c)] = res.results[c]["out"]
    return out

